# revision 4
# baseline (speedup 1.0000x reference)
"""LoftQ linear (4-bit blockwise dequant + linear + LoRA) on 8 trn2 cores.

out = x @ W^T + bias + 2.0 * (x @ A^T) @ B^T
  W[o,i] = (idx[o,i] * 2/15 - 1) * scales[o, i//64]   (idx = 4-bit nibbles)

Sharding: column-parallel — qweight/scales/bias/lora_B sharded along
out_features (4096 -> 512 per core); x and lora_A replicated; outputs
concatenated on host.

Device kernel (per core), all layouts prepared host-side:
  - contraction axis i is permuted to i' = [even i, odd i] so the nibble
    unpack of host-pre-transposed packed bytes lands in contiguous
    partition-tile halves (no on-chip transposes at all).
  - dequant: bitwise unpack (DVE) -> affine c*v-1 (ScalarE) -> *scale (DVE)
  - lora fold: W_eff = W + A'^T_chunk @ (2 B^T) via 32 small PE matmuls
  - main: 512 bf16 matmuls [K=128,M=128,N=512], psum accumulate over i',
    bias added in the psum->sbuf copy.
"""

import numpy as np
import ml_dtypes

OUT_F = 4096
IN_F = 4096
T = 2048  # 2*1024 tokens
R = 16
NCORES = 8
O_SH = OUT_F // NCORES  # 512
IPH = IN_F // 2  # 2048 packed byte-rows
C16 = 2.0 / 15.0
NQ = IPH // 128  # 16 packed tiles
NI = IN_F // 128  # 32 i' chunks
NO = O_SH // 128  # 4 o tiles
NT = T // 512  # 4 t chunks

BF16 = ml_dtypes.bfloat16

_cached = {}


def _build_nc():
    import concourse.bacc as bacc
    import concourse.mybir as mybir
    from concourse.tile import TileContext

    f32 = mybir.dt.float32
    bf16 = mybir.dt.bfloat16
    i32 = mybir.dt.int32
    AF = mybir.ActivationFunctionType
    OP = mybir.AluOpType

    nc = bacc.Bacc("TRN2", target_bir_lowering=False)

    xt = nc.dram_tensor("xt", [IN_F, T], bf16, kind="ExternalInput")
    qwt = nc.dram_tensor("qwt", [IPH, O_SH], i32, kind="ExternalInput")
    st = nc.dram_tensor("st", [IPH, O_SH], f32, kind="ExternalInput")
    apm = nc.dram_tensor("apm", [R, IN_F], bf16, kind="ExternalInput")
    bt = nc.dram_tensor("bt", [R, O_SH], bf16, kind="ExternalInput")
    bias = nc.dram_tensor("bias", [O_SH, 1], f32, kind="ExternalInput")
    out = nc.dram_tensor("out", [O_SH, T], f32, kind="ExternalOutput")

    with TileContext(nc) as tc:
        with (
            tc.tile_pool(name="w", bufs=1) as wpool,
            tc.tile_pool(name="x", bufs=2) as xpool,
            tc.tile_pool(name="cst", bufs=1) as cpool,
            tc.tile_pool(name="dq", bufs=2) as dqpool,
            tc.tile_pool(name="outp", bufs=3) as opool,
            tc.tile_pool(name="ps", bufs=3, space="PSUM") as pspool,
            tc.tile_pool(name="bps", bufs=2, space="PSUM") as bapool,
        ):
            # constants
            a_sb = cpool.tile([R, IN_F], bf16, tag="apm", name="a_sb")
            nc.sync.dma_start(out=a_sb[:], in_=apm[:])
            b_sb = cpool.tile([R, O_SH], bf16, tag="bt", name="b_sb")
            nc.sync.dma_start(out=b_sb[:], in_=bt[:])
            bias_sb = []
            for ot in range(NO):
                btile = cpool.tile([128, 1], f32, tag=f"bias{ot}", name=f"biassb{ot}")
                nc.sync.dma_start(out=btile[:], in_=bias[ot * 128 : (ot + 1) * 128, :])
                bias_sb.append(btile)

            # persistent dequantized weight tiles W_eff^T: NI x [128 i', O_SH]
            W = [wpool.tile([128, O_SH], bf16, tag=f"w{j}", name=f"wt{j}") for j in range(NI)]

            # dequant: packed tile k -> W[k] (lo nibbles) and W[NQ+k] (hi)
            for k in range(NQ):
                q = dqpool.tile([128, O_SH], i32, tag="q", name=f"q{k}")
                nc.sync.dma_start(out=q[:], in_=qwt[k * 128 : (k + 1) * 128, :])
                s = dqpool.tile([128, O_SH], f32, tag="s", name=f"s{k}")
                nc.sync.dma_start(out=s[:], in_=st[k * 128 : (k + 1) * 128, :])
                lo = dqpool.tile([128, O_SH], i32, tag="lo", name=f"lo{k}")
                nc.vector.tensor_scalar(lo[:], q[:], 15, None, OP.bitwise_and)
                hi = dqpool.tile([128, O_SH], i32, tag="hi", name=f"hi{k}")
                nc.vector.tensor_scalar(
                    hi[:], q[:], 4, 15, OP.logical_shift_right, OP.bitwise_and
                )
                ulo = dqpool.tile([128, O_SH], f32, tag="ulo", name=f"ulo{k}")
                nc.scalar.activation(ulo[:], lo[:], AF.Copy, bias=-1.0, scale=C16)
                uhi = dqpool.tile([128, O_SH], f32, tag="uhi", name=f"uhi{k}")
                nc.scalar.activation(uhi[:], hi[:], AF.Copy, bias=-1.0, scale=C16)
                nc.vector.tensor_tensor(W[k][:], ulo[:], s[:], OP.mult)
                nc.vector.tensor_tensor(W[NQ + k][:], uhi[:], s[:], OP.mult)

            # lora fold: W[j] += (A'[:, j128])^T @ (2 B^T)
            for j in range(NI):
                bp = bapool.tile([128, O_SH], f32, tag="bp", name=f"bp{j}")
                nc.tensor.matmul(
                    bp[:], a_sb[:, j * 128 : (j + 1) * 128], b_sb[:],
                    start=True, stop=True,
                )
                bsb = dqpool.tile([128, O_SH], bf16, tag="basb", name=f"bsb{j}")
                nc.scalar.activation(bsb[:], bp[:], AF.Copy)
                nc.vector.tensor_tensor(W[j][:], W[j][:], bsb[:], OP.add)

            # main matmul, streaming x by t-chunk
            for tcn in range(NT):
                X = []
                for ic in range(NI):
                    xtile = xpool.tile([128, 512], bf16, tag=f"x{ic}", name=f"xt{tcn}_{ic}")
                    nc.sync.dma_start(
                        out=xtile[:],
                        in_=xt[ic * 128 : (ic + 1) * 128, tcn * 512 : (tcn + 1) * 512],
                    )
                    X.append(xtile)
                for ot in range(NO):
                    p = pspool.tile([128, 512], f32, tag="mm", name=f"p{tcn}_{ot}")
                    for ic in range(NI):
                        nc.tensor.matmul(
                            p[:],
                            W[ic][:, ot * 128 : (ot + 1) * 128],
                            X[ic][:],
                            start=(ic == 0),
                            stop=(ic == NI - 1),
                        )
                    o_sb = opool.tile([128, 512], f32, tag="osb", name=f"osb{tcn}_{ot}")
                    nc.scalar.activation(
                        o_sb[:], p[:], AF.Identity, bias=bias_sb[ot][:], scale=1.0
                    )
                    nc.sync.dma_start(
                        out=out[ot * 128 : (ot + 1) * 128, tcn * 512 : (tcn + 1) * 512],
                        in_=o_sb[:],
                    )
    nc.compile()
    return nc


def prep_inputs(x, qweight, scales, bias, lora_A, lora_B):
    """Host-side layout prep + sharding. Returns per-core input maps."""
    x2d = np.ascontiguousarray(x.reshape(T, IN_F))
    xt = x2d.T  # [IN_F, T]
    # i' permutation: even original i first, then odd
    xp = np.ascontiguousarray(
        np.concatenate([xt[0::2], xt[1::2]], axis=0)
    ).astype(BF16)

    ap = np.ascontiguousarray(
        np.concatenate([lora_A[:, 0::2], lora_A[:, 1::2]], axis=1)
    ).astype(BF16)

    qw2 = qweight.reshape(OUT_F, IPH)  # byte (o, ip) holds i=2ip (lo), 2ip+1 (hi)
    sc2 = scales.reshape(OUT_F, IN_F // 64)

    in_maps = []
    for c in range(NCORES):
        o0, o1 = c * O_SH, (c + 1) * O_SH
        qwt_c = np.ascontiguousarray(qw2[o0:o1].T).astype(np.int32)  # [IPH, O_SH]
        # scale for (ip, o) = scales[o, ip//32] (same for lo and hi nibble)
        st_c = np.ascontiguousarray(
            np.repeat(sc2[o0:o1].T, 32, axis=0)
        ).astype(np.float32)  # [IPH, O_SH]
        bt_c = np.ascontiguousarray(2.0 * lora_B[o0:o1].T).astype(BF16)  # [R, O_SH]
        bias_c = np.ascontiguousarray(bias[o0:o1].reshape(O_SH, 1)).astype(np.float32)
        in_maps.append(
            {"xt": xp, "qwt": qwt_c, "st": st_c, "apm": ap, "bt": bt_c, "bias": bias_c}
        )
    return in_maps


def run(in_maps, trace=False):
    from concourse import bass_utils

    if "nc" not in _cached:
        _cached["nc"] = _build_nc()
    res = bass_utils.run_bass_kernel_spmd(
        _cached["nc"], in_maps, list(range(NCORES)), trace=trace
    )
    return res


def assemble(results):
    full = np.concatenate(
        [np.asarray(r["out"], dtype=np.float32) for r in results], axis=0
    )  # [OUT_F, T]
    return np.ascontiguousarray(full.T).reshape(2, 1024, OUT_F)


def kernel(x, qweight, scales, bias, lora_A, lora_B):
    in_maps = prep_inputs(x, qweight, scales, bias, lora_A, lora_B)
    res = run(in_maps, trace=False)
    return assemble(res.results)
